# revision 14
# baseline (speedup 1.0000x reference)
"""Trainium2 Bass kernel for nn_MultiHeadGraphAttention.

Reference computation (B=4, N=2048, D=256, H=8, DK=32):
    Q = x @ w_q.T ; K = x @ w_k.T ; V = x @ w_v.T        (split into 8 heads of 32)
    scores = (Q K^T)/sqrt(32) + edge_weights, masked where mask==0
    out = softmax(scores) V  -> merge heads -> @ w_o.T + b_o
Sharding: 8 cores = batch(4) x sequence-halves(2). Each core owns batch b,
rows n0..n0+1023 and produces the full [1024, 256] output slab for them.

Per-core algorithm (transposed layout [feature/key, seq]):
  - w_q is prescaled by A/sqrt(32) with A = 128*log2(e), so the QK^T matmul
    lands A*s in PSUM.
  - Ae = A*edge (masked entries = -1e9) is precomputed on host in bf16.
  - Score tiles are drained from PSUM by one of two paths, split to balance
    the Scalar and Vector engines:
      S path: PE adds Ae into PSUM (identity-weight accumulating matmul),
              then ScalarE ACT computes exp(x/A) -> bf16 numerator.
      D path: one DVE scalar_tensor_tensor (PSUM + B) + Ae -> int16, whose
              bit pattern reinterpreted as bf16 IS ~exp(s+e) (Schraudolph in
              bf16: y = A*(s+e) + 16256 - C => bitcast ~ 2^(y/128-127)).
              Masked entries saturate to -32768 = -0.0 in bf16.
  - attention @ V via PE with V augmented by a ones column, yielding the
    softmax denominators as extra PSUM rows; 1/denom via ScalarE ln then
    exp(-x) (same ACT table set); normalization applied to the small
    [256,1024] head outputs.
"""

import sys

for _p in ("/opt/trn_rl_repo", "/root/.axon_site/_ro/trn_rl_repo"):
    if _p not in sys.path:
        sys.path.insert(0, _p)

import numpy as np
import ml_dtypes

import concourse.bass as bass
import concourse.mybir as mybir
import concourse.tile as tile
from concourse.bass_utils import run_bass_kernel_spmd

B, N, D, H, DK = 4, 2048, 256, 8, 32
NL = N // 2          # rows per core
SCALE = float(np.sqrt(DK))
MB = N // 128        # 16 key blocks
NCH = NL // 512      # 2 query chunks of 512
F32 = mybir.dt.float32
BF16 = mybir.dt.bfloat16
I16 = mybir.dt.int16
I32 = mybir.dt.int32

A_SCHR = 128.0 / float(np.log(2.0))          # 184.6650
B_SCHR = 16256.0 - 7.3                        # 127*128 - C, C tuned for 0 bias
NEG_BIG = -1.0e9
# Per-grp drain path: 'S' = PE edge-add + ScalarE exp, 'D' = DVE Schraudolph.
def paths_for(nch, mb):
    return ("S", "D", "S", "D")

_wait_ctr = [0]


def _split_multi_waits(nc, max_waits=1):
    """Walrus in this container rejects >1 sync wait per instruction; move
    extra waits onto NOPs inserted just before, on the same engine."""
    for fn in nc.m.functions:
        for bb in fn.blocks:
            insts = bb.instructions
            out = []
            changed = False
            for inst in insts:
                si = inst.sync_info
                if si is not None and len(si.on_wait) > max_waits:
                    waits = list(si.on_wait)
                    for w in waits[:-max_waits]:
                        _wait_ctr[0] += 1
                        out.append(
                            mybir.InstNoOp(
                                name=f"waitsplit-nop-{_wait_ctr[0]}",
                                engine=inst.engine,
                                sync_info=mybir.SyncInfo(on_wait=[w], on_update=[]),
                            )
                        )
                    inst.sync_info = mybir.SyncInfo(
                        on_wait=waits[-max_waits:], on_update=list(si.on_update)
                    )
                    changed = True
                out.append(inst)
            if changed:
                insts.clear()
                insts.extend(out)


def _build_program():
    nc = bass.Bass()

    xT = nc.dram_tensor("xT", [D, N], F32, kind="ExternalInput")
    xTq = nc.dram_tensor("xTq", [D, NL], F32, kind="ExternalInput")
    aeT = nc.dram_tensor("aeT", [N, NL], BF16, kind="ExternalInput")
    wqT = nc.dram_tensor("wqT", [D, D], F32, kind="ExternalInput")
    wkT = nc.dram_tensor("wkT", [D, D], F32, kind="ExternalInput")
    wvT = nc.dram_tensor("wvT", [D, D], F32, kind="ExternalInput")
    woT = nc.dram_tensor("woT", [D, D], F32, kind="ExternalInput")
    bo = nc.dram_tensor("bo", [1, D], F32, kind="ExternalInput")
    ident = nc.dram_tensor("ident", [128, 128], BF16, kind="ExternalInput")
    outd = nc.dram_tensor("out", [NL, D], F32, kind="ExternalOutput")

    with tile.TileContext(nc) as tc:
        with (
            tc.tile_pool(name="singles", bufs=1) as singles,
            tc.tile_pool(name="persist", bufs=1) as persist,
        ):
            # A*edge tiles, one per key block (bf16, masked entries -1e9)
            ae_sb_early = [persist.tile([128, NL], BF16, name=f"ae{mb}")
                           for mb in range(MB)]
            # ---- static tiles -------------------------------------------------
            xT_sb = [singles.tile([128, N], BF16, name=f"xt{p}") for p in range(2)]
            xTq_sb = [singles.tile([128, NL], BF16, name=f"xtq{p}") for p in range(2)]
            wq_sb = [singles.tile([128, D], BF16, name=f"wq{p}") for p in range(2)]
            wk_sb = [singles.tile([128, D], BF16, name=f"wk{p}") for p in range(2)]
            wv_sb = [singles.tile([128, D], BF16, name=f"wv{p}") for p in range(2)]
            wo_sb = [singles.tile([128, D], BF16, name=f"wo{p}") for p in range(2)]
            bo_sb = singles.tile([128, D], F32, name="bo_sb")
            ones128 = singles.tile([128, 32], BF16, name="ones128")
            nc.vector.memset(ones128[:], 1.0)
            id_sb = singles.tile([128, 128], BF16, name="id_sb")
            nc.gpsimd.dma_start(out=id_sb[:], in_=ident[:, :])

            xTf = [singles.tile([128, N], F32, name=f"xtf{p}") for p in range(2)]
            xTqf = [singles.tile([128, NL], F32, name=f"xtqf{p}") for p in range(2)]
            for mb0 in range(2):
                nc.sync.dma_start(
                    out=ae_sb_early[mb0][:], in_=aeT[mb0 * 128:(mb0 + 1) * 128, :]
                )
            for p in range(2):
                nc.sync.dma_start(out=xTqf[p][:], in_=xTq[p * 128:(p + 1) * 128, :])
                nc.sync.dma_start(out=xTf[p][:], in_=xT[p * 128:(p + 1) * 128, :])
                nc.vector.tensor_copy(xTq_sb[p][:], xTqf[p][:])
                nc.vector.tensor_copy(xT_sb[p][:], xTf[p][:])
                nc.gpsimd.dma_start(out=wq_sb[p][:], in_=wqT[p * 128:(p + 1) * 128, :])
                nc.gpsimd.dma_start(out=wk_sb[p][:], in_=wkT[p * 128:(p + 1) * 128, :])
                nc.gpsimd.dma_start(out=wv_sb[p][:], in_=wvT[p * 128:(p + 1) * 128, :])
                nc.gpsimd.dma_start(out=wo_sb[p][:], in_=woT[p * 128:(p + 1) * 128, :])
            nc.gpsimd.dma_start(out=bo_sb[:], in_=bo[0:1, :].partition_broadcast(128))

            # persistent intermediates (Q/K in bf16: halves PE stream cost)
            QT_sb = [persist.tile([128, NL], BF16, name=f"qt{p}") for p in range(2)]
            KT_sb = [persist.tile([128, N], BF16, name=f"kt{p}") for p in range(2)]
            # V augmented with a ones column: AV matmul (M=33) then yields both
            # attention@V (rows 0-31) and the softmax denominator (row 32).
            V_aug = [persist.tile([128, H, 33], BF16, name=f"v_aug{mb}")
                     for mb in range(MB)]
            for mb in range(MB):
                nc.gpsimd.memset(V_aug[mb][:, :, 32:33], 1.0)
            ae_sb = ae_sb_early
            houtT = [[persist.tile([128, 512], BF16, name=f"ho{g}_{c}")
                      for c in range(NCH)] for g in range(2)]

            # ---- attention main loop -----------------------------------------
            with (
                tc.tile_pool(name="spool", bufs=2, space="PSUM") as spool,
                tc.tile_pool(name="avpool", bufs=1, space="PSUM") as avpool,
                tc.tile_pool(name="numpool", bufs=28) as numpool,
                tc.tile_pool(name="rcppool", bufs=2) as rcppool,
                tc.tile_pool(name="outpool", bufs=3) as outpool,
            ):
                def q_proj(p, f):
                    qps = spool.tile([128, 512], F32, name="qps", tag="s")
                    for dc in range(2):
                        nc.tensor.matmul(
                            qps[:],
                            wq_sb[dc][:, p * 128:(p + 1) * 128],
                            xTq_sb[dc][:, f * 512:(f + 1) * 512],
                            start=(dc == 0), stop=(dc == 1),
                        )
                    nc.scalar.copy(
                        QT_sb[p][:, f * 512:(f + 1) * 512], qps[:]
                    )

                def k_proj(p, f):
                    kps = spool.tile([128, 512], F32, name="kps", tag="s")
                    for dc in range(2):
                        nc.tensor.matmul(
                            kps[:],
                            wk_sb[dc][:, p * 128:(p + 1) * 128],
                            xT_sb[dc][:, f * 512:(f + 1) * 512],
                            start=(dc == 0), stop=(dc == 1),
                        )
                    nc.scalar.copy(
                        KT_sb[p][:, f * 512:(f + 1) * 512], kps[:]
                    )

                def v_proj(mb):
                    vps = spool.tile([128, D], F32, name="vps", tag="s")
                    for dc in range(2):
                        nc.tensor.matmul(
                            vps[:],
                            xT_sb[dc][:, mb * 128:(mb + 1) * 128],
                            wv_sb[dc][:],
                            start=(dc == 0), stop=(dc == 1),
                        )
                    nc.vector.tensor_copy(
                        V_aug[mb][:, :, 0:32],
                        vps[:].rearrange("p (h d) -> p h d", h=H),
                    )

                def ae_fetch(mb):
                    eng = nc.sync if mb % 2 == 0 else nc.gpsimd
                    eng.dma_start(
                        out=ae_sb[mb][:], in_=aeT[mb * 128:(mb + 1) * 128, :]
                    )

                for p in range(2):
                    for f in range(NCH):
                        q_proj(p, f)
                    k_proj(p, 0)
                for mb in range(4):
                    v_proj(mb)
                AE_PF = 4
                for mb in range(2, AE_PF):
                    ae_fetch(mb)
                pending = []
                for nch in range(NCH):
                    nsl = slice(nch * 512, (nch + 1) * 512)
                    # bank b holds heads (2b, 2b+1): rows 0-32 and 64-96
                    avps = [
                        avpool.tile([128, 512], F32, name=f"av{b}", tag=f"av{b}")
                        for b in range(4)
                    ]
                    avq = []

                    def av_issue(item, avps=avps):
                        mb_i, grp_i, nrhs = item
                        for hh2 in range(2):
                            h = grp_i * 2 + hh2
                            b, sub = h // 2, h % 2
                            nc.tensor.matmul(
                                avps[b][64 * sub:64 * sub + 33, :],
                                V_aug[mb_i][:, h, 0:33],
                                nrhs[:, hh2 * 512:(hh2 + 1) * 512],
                                start=(mb_i == 0), stop=(mb_i == MB - 1),
                                tile_position=(0, 64 * sub),
                            )

                    for mb in range(MB):
                        if nch == 0:
                            if mb % 4 == 2 and mb // 4 + 1 < 4:
                                k_proj(0, mb // 4 + 1)
                                k_proj(1, mb // 4 + 1)
                            if mb + 4 < MB:
                                v_proj(mb + 4)
                            if mb + AE_PF < MB:
                                ae_fetch(mb + AE_PF)
                        if nch == 1 and mb == 2 and pending:
                            pending.pop(0)()
                        for grp in range(4):  # 2 heads per group
                            path = paths_for(nch, mb)[grp]
                            sps = spool.tile([128, 1024], F32, name="sps", tag="s")
                            for hh2 in range(2):
                                h = grp * 2 + hh2
                                # scores_T[m,n] = sum_dk K_T[dk,m] * Q_T[dk,n]
                                nc.tensor.matmul(
                                    sps[:, hh2 * 512:(hh2 + 1) * 512],
                                    KT_sb[h // 4][(h % 4) * 32:(h % 4 + 1) * 32,
                                                  mb * 128:(mb + 1) * 128],
                                    QT_sb[h // 4][(h % 4) * 32:(h % 4 + 1) * 32, nsl],
                                    start=True, stop=(path == "D"),
                                    tile_position=(32 * (h % 4), 0),
                                )
                            if path == "S":
                                # PSUM += A*edge via identity-weight matmul,
                                # then exp(x/A) on ScalarE
                                for hh2 in range(2):
                                    nc.tensor.matmul(
                                        sps[:, hh2 * 512:(hh2 + 1) * 512],
                                        id_sb[:],
                                        ae_sb[mb][:, nsl],
                                        start=False, stop=True,
                                    )
                                numer = numpool.tile(
                                    [128, 1024], BF16, name="numer", tag="n"
                                )
                                nc.scalar.activation(
                                    numer[:], sps[:],
                                    mybir.ActivationFunctionType.Exp,
                                    bias=0.0, scale=1.0 / A_SCHR,
                                )
                                nrhs = numer
                            else:
                                # Schraudolph: int16(A*s + B + A*e) bitcast bf16
                                numer_i = numpool.tile(
                                    [128, 1024], I16, name="numer", tag="n"
                                )
                                for hh2 in range(2):
                                    nc.vector.scalar_tensor_tensor(
                                        numer_i[:, hh2 * 512:(hh2 + 1) * 512],
                                        sps[:, hh2 * 512:(hh2 + 1) * 512],
                                        B_SCHR,
                                        ae_sb[mb][:, nsl],
                                        mybir.AluOpType.add,
                                        mybir.AluOpType.add,
                                    )
                                nrhs = numer_i.bitcast(BF16)
                            avq.append((mb, grp, nrhs))
                            # hold nch1's AVs while norm_nch0 drains the
                            # previous PSUM accumulators, so the PE stays
                            # busy (HAM stays warm) across the boundary
                            hold = nch == 1 and mb < 4
                            if not hold:
                                while len(avq) > 2:
                                    av_issue(avq.pop(0))
                    while avq:
                        av_issue(avq.pop(0))
                    # ln(denominators) straight from PSUM, and remap the AV
                    # blocks PSUM->SBUF bf16 so head hg*4+j lands at rows
                    # 32j (PSUM reads allow a partition offset; SBUF-SBUF
                    # ops require matching bases). Normalization then is a
                    # single full-tile bf16 multiply per head group.
                    avcp = [
                        outpool.tile([128, 512], BF16, name=f"avcp{hg}",
                                     tag=f"avcp{hg}")
                        for hg in range(2)
                    ]
                    lntmp = [
                        rcppool.tile([128, 512], F32, name=f"lntmp{b}",
                                     tag=f"lntmp{b % 2}")
                        for b in range(4)
                    ]
                    for b in range(4):
                        nc.scalar.activation(
                            lntmp[b][0:97, :], avps[b][0:97, :],
                            mybir.ActivationFunctionType.Ln,
                            bias=0.0, scale=1.0,
                        )
                    for hg in range(2):
                        for j in range(4):
                            h = hg * 4 + j
                            b, sub = h // 2, h % 2
                            nc.vector.tensor_copy(
                                avcp[hg][32 * j:32 * j + 32, :],
                                avps[b][64 * sub:64 * sub + 32, :],
                            )

                    def norm_and_proj(nch=nch, avcp=avcp, lntmp=lntmp):
                        rcpx = [
                            rcppool.tile([128, 512], BF16, name=f"rcpx{b}",
                                         tag=f"rcpx{b % 2}")
                            for b in range(4)
                        ]
                        for b in range(4):
                            nc.scalar.activation(
                                rcpx[b][0:97, :], lntmp[b][0:97, :],
                                mybir.ActivationFunctionType.Exp,
                                bias=0.0, scale=-1.0,
                            )
                        for hg in range(2):
                            rcpb_ps = spool.tile(
                                [128, 512], F32, name="rcpb_ps", tag="s"
                            )
                            for j in range(4):
                                h = hg * 4 + j
                                b, sub = h // 2, h % 2
                                nc.tensor.matmul(
                                    rcpb_ps[32 * j:32 * j + 32, :],
                                    ones128[64 * sub + 32:64 * sub + 33, 0:32],
                                    rcpx[b][64 * sub + 32:64 * sub + 33, :],
                                    start=True, stop=True,
                                    tile_position=(64 * sub + 32, 32 * j),
                                )
                            rcpb_g = rcppool.tile(
                                [128, 512], BF16, name=f"rcpb{hg}", tag=f"rcpb{hg}"
                            )
                            nc.vector.tensor_copy(rcpb_g[:], rcpb_ps[:])
                            nc.vector.tensor_mul(
                                houtT[hg][nch][:], avcp[hg][:], rcpb_g[:]
                            )
                        for nbl in range(4):
                            nb = nch * 4 + nbl
                            ops = spool.tile([128, D], F32, name="ops", tag="s")
                            for g in range(2):
                                nc.tensor.matmul(
                                    ops[:],
                                    houtT[g][nch][:, nbl * 128:(nbl + 1) * 128],
                                    wo_sb[g][:],
                                    start=(g == 0), stop=(g == 1),
                                )
                            osb = outpool.tile([128, D], F32, name="osb", tag="osb")
                            nc.vector.tensor_add(osb[:], ops[:], bo_sb[:])
                            nc.sync.dma_start(
                                out=outd[nb * 128:(nb + 1) * 128, :], in_=osb[:]
                            )

                    pending.append(norm_and_proj)
                for fn in pending:
                    fn()

    _split_multi_waits(nc)
    return nc


_NC_CACHE = None


def _get_program():
    global _NC_CACHE
    if _NC_CACHE is None:
        _NC_CACHE = _build_program()
    return _NC_CACHE


def _make_in_maps(x, edge_weights, mask, w_q, w_k, w_v, w_o, b_o):
    wqT = np.ascontiguousarray((w_q * (A_SCHR / SCALE)).T).astype(np.float32)
    wkT = np.ascontiguousarray(w_k.T).astype(np.float32)
    wvT = np.ascontiguousarray(w_v.T).astype(np.float32)
    woT = np.ascontiguousarray(w_o.T).astype(np.float32)
    bo = np.ascontiguousarray(b_o.reshape(1, D)).astype(np.float32)
    ident = np.eye(128, dtype=ml_dtypes.bfloat16)
    in_maps = []
    for c in range(8):
        b, half = c // 2, c % 2
        n0 = half * NL
        xTb = np.ascontiguousarray(x[b].T).astype(np.float32)
        ae = np.where(
            mask[b, n0:n0 + NL, :] != 0,
            A_SCHR * edge_weights[b, n0:n0 + NL, :],
            np.float32(NEG_BIG),
        ).T.astype(ml_dtypes.bfloat16)
        in_maps.append({
            "xT": xTb,
            "xTq": np.ascontiguousarray(xTb[:, n0:n0 + NL]),
            "aeT": np.ascontiguousarray(ae),
            "wqT": wqT, "wkT": wkT, "wvT": wvT, "woT": woT, "bo": bo,
            "ident": ident,
        })
    return in_maps


def run_sharded(inputs, trace=False, tmpdir=None):
    """Run the SPMD kernel; returns (full_output, BassKernelResults)."""
    arrs = {k: np.asarray(v) for k, v in inputs.items()}
    nc = _get_program()
    in_maps = _make_in_maps(**arrs)
    res = run_bass_kernel_spmd(
        nc, in_maps, list(range(8)), trace=trace, tmpdir=tmpdir
    )
    out = np.empty((B, N, D), np.float32)
    for c in range(8):
        b, half = c // 2, c % 2
        out[b, half * NL:(half + 1) * NL, :] = res.results[c]["out"]
    return out, res


def kernel(**inputs):
    out, _ = run_sharded(inputs, trace=False)
    return out
